# revision 9
# baseline (speedup 1.0000x reference)
"""Trainium2 Bass kernel for nn_PixelWiseAdpNet.

Sharding: (batch=4) x (patch-row-half=2) -> 8 cores; each core owns one
batch's 4x8 block of patches (32 patches, 16384 points).

v2 changes vs baseline (246us):
  - W2 region of w_feat streamed as fp8 e3m4 (x256 scale folded into a
    second copy of the patch-feature rhs); W1/W3/biases and all other
    tensors fp16 (not bf16) to buy numeric headroom for the fp8. The
    w_feat stream drops 46.5MB -> 29.7MB per core.
  - em merged into the coord-data matmul (augmented [w_cd.T; b_cd; I32]
    contraction, K=97) so inX needs one PSUM->SBUF copy, no add.
  - L1 bias folded into the matmul via a ones row (K=33; the W1''
    stream region interleaves b1 as hyper-channel i=32), so L1's
    LeakyReLU is bias-free and runs as ONE activation per patch pair.
  - Batched DMAs (>=4KB descriptors, r-major dram layouts for coords
    and output): ~45 triggers total vs ~170.
Stream order: [head fp16: b2 b3 W1''][W2 fp8e3][W3 fp16]; L1/L2h0/L2h1
interleave with the stream as their param regions complete; L3 after W3.
"""

import numpy as np
import ml_dtypes

import concourse.mybir as mybir
import concourse.tile as tile
from concourse import bacc
from concourse.bass_utils import run_bass_kernel_spmd
from concourse.masks import make_identity

F16 = np.float16
E3 = ml_dtypes.float8_e3m4

B, IN_CH, OUT_CH, FEAT_CH = 4, 32, 64, 256
AH = AW = 8
OUT_H = OUT_W = 64
S = 8
NEG = 0.01
N_CORES = 8
NQ = 32                  # patches per core

# stream tile layout (128-c tiles)
#   head (fp16): [b2:0-1][b3+pad:2][W1'':3-68]  (69 tiles, 8832 c)
#   w2   (fp8) : 512 tiles (65536 c)
#   w3   (fp16): 128 tiles (16384 c)
N_HEAD, N_W2, N_W3 = 69, 512, 128
N_TILES = N_HEAD + N_W2 + N_W3          # 709
T_W2, T_W3 = N_HEAD, N_HEAD + N_W2      # global region starts
W2_SCALE = 256.0

FP = mybir.dt.float16
F32 = mybir.dt.float32
F8 = mybir.dt.float8e3

_CACHE = {}


def _build(variant="all"):
    nreps = 1
    if variant.startswith("rep"):
        nreps = int(variant[3:])
        variant = "all"
    nc = bacc.Bacc("TRN2", target_bir_lowering=False, debug=False,
                   num_devices=N_CORES)

    whead_d = nc.dram_tensor("whead", [128, 2, N_HEAD * 128], FP,
                             kind="ExternalInput")
    w2_d = nc.dram_tensor("w2", [128, 2, N_W2 * 128], F8, kind="ExternalInput")
    w3_d = nc.dram_tensor("w3", [128, 2, N_W3 * 128], FP, kind="ExternalInput")
    bfeat_d = nc.dram_tensor("bfeat", [128, N_TILES], F32, kind="ExternalInput")
    mlpfT_d = nc.dram_tensor("mlpfT", [128, 2, NQ], FP, kind="ExternalInput")
    mlpfs_d = nc.dram_tensor("mlpfs", [128, 2, NQ], FP, kind="ExternalInput")
    cdem_d = nc.dram_tensor("cdem", [97, 4, S, 8, 64], FP, kind="ExternalInput")
    wcdT_d = nc.dram_tensor("wcdT", [97, IN_CH], FP, kind="ExternalInput")
    ones_d = nc.dram_tensor("ones", [1, 4, S, 8, 64], FP, kind="ExternalInput")
    out_d = nc.dram_tensor("out", [OUT_CH, 8, S, 8, 32], F32,
                           kind="ExternalOutput")

    # slab schedule: (dram, global tile0, ntiles, scaled_rhs)
    slabs = []
    for t0 in range(0, N_HEAD, 16):
        slabs.append((whead_d, t0, min(16, N_HEAD - t0), False))
    for t0 in range(0, N_W2, 32):
        slabs.append((w2_d, T_W2 + t0, 32, True))
    for t0 in range(0, N_W3, 16):
        slabs.append((w3_d, T_W3 + t0, 16, False))
    N_SL_HEAD = (N_HEAD + 15) // 16                    # 5
    SL_W2_END = N_SL_HEAD + N_W2 // 32                 # 21
    N_SLABS = len(slabs)                               # 29
    region_t0 = {id(whead_d): 0, id(w2_d): T_W2, id(w3_d): T_W3}

    with tile.TileContext(nc) as tc:
        with (
            tc.tile_pool(name="const", bufs=1) as const_pool,
            tc.tile_pool(name="wstream", bufs=3) as wpool,
            tc.tile_pool(name="params", bufs=1) as ppool,
            tc.tile_pool(name="acts", bufs=3) as apool,
            tc.tile_pool(name="psA", bufs=2, space="PSUM") as psA,
            tc.tile_pool(name="psM", bufs=2, space="PSUM") as psM,
            tc.tile_pool(name="psY", bufs=2, space="PSUM") as psY,
        ):
            for _rep in range(nreps):
                # ---- constants ----
                ident = const_pool.tile([128, 128], FP, name="ident")
                make_identity(nc, ident[:])
                mlpfT = const_pool.tile([128, 2, NQ], FP, name="mlpfT")
                nc.scalar.dma_start(mlpfT[:], mlpfT_d[:])
                mlpfs = const_pool.tile([128, 2, NQ], FP, name="mlpfs")
                nc.scalar.dma_start(mlpfs[:], mlpfs_d[:])
                wcdT = const_pool.tile([97, IN_CH], FP, name="wcdT")
                nc.scalar.dma_start(wcdT[:], wcdT_d[:])
                bfeat = const_pool.tile([128, N_TILES], F32, name="bfeat")
                nc.scalar.dma_start(bfeat[:], bfeat_d[:])

                REG = [(0, T_W2), (T_W2, T_W3), (T_W3, N_TILES)]
                pregs = [
                    ppool.tile([128, N_HEAD, NQ], FP, name="pW1"),
                    ppool.tile([128, N_W2, NQ], FP, name="pW2"),
                    ppool.tile([128, N_W3, NQ], FP, name="pW3"),
                ]
                # bias_sb: 0,1 = b2 halves (head tiles 0,1); 2 = b3 (tile 2)
                bias_sb = ppool.tile([128, 3, NQ], F32, name="bias_sb")

                def preg(t):
                    for (lo, hi), pt in zip(REG, pregs):
                        if lo <= t < hi:
                            return pt, t - lo
                    raise AssertionError(t)

                if variant == "noA":
                    for pt in pregs:
                        nc.vector.memset(pt[:], 0.0)
                    nc.vector.memset(bias_sb[:], 0.0)
                n_slabs = 0 if variant == "noA" else N_SLABS

                pend = {}
                cursor = {"dma": 0, "go": 0}

                def slab_dma():
                    sl = cursor["dma"]
                    if sl >= n_slabs:
                        return
                    cursor["dma"] += 1
                    dram, t0, nt, _sc = slabs[sl]
                    cw = nt * 128
                    c0 = (t0 - region_t0[id(dram)]) * 128
                    dt = F8 if dram is w2_d else FP
                    wbuf = wpool.tile([128, 2, 4096 if dt is F8 else 2048],
                                      dt, name="wbuf")
                    nc.sync.dma_start(wbuf[:, :, :cw], dram[:, :, c0:c0 + cw])
                    pend[sl] = wbuf

                def slab_go():
                    sl = cursor["go"]
                    if sl >= n_slabs or sl not in pend:
                        return
                    cursor["go"] += 1
                    dram, t0, nt, scaled = slabs[sl]
                    wbuf = pend.pop(sl)
                    rhs = mlpfs if scaled else mlpfT
                    for ch0 in range(0, nt, 16):
                        chn = min(16, nt - ch0)
                        ps = psA.tile([128, 16, NQ], F32, name="ps")
                        for u in range(chn):
                            for k in range(2):
                                nc.tensor.matmul(
                                    ps[:, u, :],
                                    wbuf[:, k, (ch0 + u) * 128:(ch0 + u + 1) * 128],
                                    rhs[:, k, :],
                                    start=(k == 0), stop=(k == 1))
                        u = 0
                        while u < chn:
                            t = t0 + ch0 + u
                            pt, lt = preg(t)
                            seg = min(chn - u,
                                      next(hi for (lo, hi) in REG if lo <= t < hi) - t)
                            nc.vector.tensor_tensor(
                                out=pt[:, lt:lt + seg, :],
                                in0=ps[:, u:u + seg, :],
                                in1=bfeat[:, t:t + seg].unsqueeze(2).broadcast_to(
                                    (128, seg, NQ)),
                                op=mybir.AluOpType.add)
                            u += seg
                        for t in range(3):
                            if t0 + ch0 <= t < t0 + ch0 + chn:
                                nc.vector.tensor_scalar_add(
                                    bias_sb[:, t, :], ps[:, t - t0 - ch0, :],
                                    bfeat[:, t:t + 1])

                mlp_on = variant != "nomlp"
                x1s, x2h0s, x2h1s = {}, {}, {}

                with tc.tile_pool(name="early", bufs=1) as epool:
                    # ---- phase inX ----
                    x0 = epool.tile([33, 4, S, 8, 64], FP, name="x0")
                    nc.scalar.dma_start(x0[32:33, :, :, :, :], ones_d[:])
                    cdem_ts = {}

                    def emit_inx(r, s):
                        cd_ps = psM.tile([IN_CH, 8, 64], F32, name="cd_ps",
                                         tag="cd", bufs=2)
                        nc.tensor.matmul(cd_ps[:], wcdT[:],
                                         cdem_ts[r][:, s, :, :],
                                         start=True, stop=True)
                        nc.vector.tensor_copy(x0[:32, r, s, :, :], cd_ps[:])

                    for r in range(4):
                        t = epool.tile([97, S, 8, 64], FP, name="cdem_t",
                                       bufs=2)
                        nc.scalar.dma_start(t[:], cdem_d[:, r, :, :, :])
                        cdem_ts[r] = t

                    slab_dma()
                    for sl in range(N_SL_HEAD):
                        slab_dma()
                        if sl < 4:
                            for s in range(S):
                                emit_inx(sl, s)
                        slab_go()

                    if mlp_on:
                        # W1'' fixup + L1 for all pairs; W2h0 stream drains
                        # in the gaps (8 slabs over 32 pairs)
                        for q in range(NQ):
                            r, wp = q // 8, q % 8
                            w1T = epool.tile([33, 256], FP, name="w1T", bufs=6)
                            tp = psY.tile([128, 2, 512], FP, name="tp",
                                          tag="y")
                            for h in range(2):
                                nc.tensor.transpose(
                                    tp[:33, 0, 128 * h:128 * h + 128],
                                    pregs[0][:, 3 + h:69:2, q],
                                    ident[:])
                            nc.vector.tensor_copy(w1T[:], tp[:33, 0, :256])
                            xq = x0[:, r, :, :, 8 * wp:8 * wp + 8]
                            x1 = apool.tile([128, 2, 512], FP, name="x1",
                                            bufs=NQ)
                            y1 = psY.tile([128, 2, 512], F32, name="y1",
                                          tag="y")
                            for h in range(2):
                                nc.tensor.matmul(
                                    y1[:, h, :], w1T[:, 128 * h:128 * h + 128],
                                    xq, start=True, stop=True)
                            nc.scalar.activation(
                                x1[:, :, :], y1[:, :, :],
                                mybir.ActivationFunctionType.Lrelu,
                                bias=0.0, scale=1.0, alpha=NEG)
                            x1s[q] = x1
                            if q % 4 == 3:
                                slab_dma()
                                slab_go()

                with tc.tile_pool(name="late", bufs=1) as lpool:
                    if mlp_on:
                        # L2 h=0 (W2 local tiles 0..255 ready); W2h1 stream
                        # drains in the gaps
                        for q in range(NQ):
                            x2 = lpool.tile([128, 512], FP, name="x2h0",
                                            bufs=NQ)
                            y2 = psY.tile([128, 2, 512], F32, name="y2",
                                          tag="y")
                            for k in range(2):
                                nc.tensor.matmul(
                                    y2[:, 0, :], pregs[1][:, k:k + 255:2, q],
                                    x1s[q][:, k, :], start=(k == 0),
                                    stop=(k == 1))
                            nc.scalar.activation(
                                x2[:], y2[:, 0, :],
                                mybir.ActivationFunctionType.Lrelu,
                                bias=bias_sb[:, 0, q:q + 1], scale=1.0,
                                alpha=NEG)
                            x2h0s[q] = x2
                            if q % 4 == 1:
                                slab_dma()
                            if q % 4 == 3:
                                slab_go()
                    while cursor["go"] < SL_W2_END and cursor["go"] < n_slabs:
                        slab_dma()
                        slab_go()

                    H1E = 20

                    def emit_l2h1(q):
                        x2 = lpool.tile([128, 512], FP, name="x2h1", bufs=H1E)
                        y2 = psY.tile([128, 2, 512], F32, name="y2", tag="y")
                        for k in range(2):
                            t0 = 256 + k
                            nc.tensor.matmul(
                                y2[:, 0, :], pregs[1][:, t0:t0 + 255:2, q],
                                x1s[q][:, k, :], start=(k == 0), stop=(k == 1))
                        nc.scalar.activation(
                            x2[:], y2[:, 0, :],
                            mybir.ActivationFunctionType.Lrelu,
                            bias=bias_sb[:, 1, q:q + 1], scale=1.0, alpha=NEG)
                        x2h1s[q] = x2

                    out_state = {}

                    def emit_l3(q):
                        r, wp = q // 8, q % 8
                        hr = 2 * r + wp // 4
                        if wp % 4 == 0:
                            out_state[hr] = lpool.tile([OUT_CH, S, 8, 32], F32,
                                                       name="out_row", bufs=2)
                        y3 = psY.tile([128, 2, 512], F32, name="y3", tag="y")
                        x2h = [x2h0s[q], x2h1s[q]]
                        for k in range(2):
                            nc.tensor.matmul(
                                y3[:OUT_CH, 0, :], pregs[2][:, k:k + 127:2, q],
                                x2h[k][:], start=(k == 0), stop=(k == 1))
                        wo = (wp % 4) * 8
                        nc.vector.tensor_scalar_add(
                            out_state[hr][:, :, :, wo:wo + 8],
                            y3[:OUT_CH, 0, :], bias_sb[:OUT_CH, 2, q:q + 1])
                        if wp % 4 == 3:
                            nc.sync.dma_start(out_d[:, hr, :, :, :],
                                              out_state[hr][:])

                    # W3 stream overlaps L2h1; L3 tail after W3 lands
                    for i in range(8):
                        slab_dma()
                        if mlp_on:
                            for q in range(2 * i, 2 * i + 2):
                                if q < H1E:
                                    emit_l2h1(q)
                        slab_go()
                    if mlp_on:
                        for q in range(16, H1E):
                            emit_l2h1(q)

                    if variant == "nomlp":
                        for hr in range(8):
                            out_row = lpool.tile([OUT_CH, S, 8, 32], F32,
                                                 name="out_rowM", bufs=2)
                            nc.vector.memset(out_row[:], 0.0)
                            nc.sync.dma_start(out_d[:, hr, :, :, :],
                                              out_row[:])
                    if mlp_on:
                        # L3(i) frees the x2h1 slot L2h1(i+H1E) needs
                        for q in range(H1E, NQ):
                            emit_l3(q - H1E)
                            emit_l2h1(q)
                        for q in range(NQ - H1E, NQ):
                            emit_l3(q)

    nc.compile()
    return nc


def _host_prep(MLP_feature, coord_em, coord_data, w_cd, b_cd, w_feat, b_feat):
    # orig c ranges: W1 0..8191 (c=o*32+i), b1 8192..8447, W2 8448..73983,
    # b2 73984..74239, W3 74240..90623, b3 90624..90687
    C_HEAD = N_HEAD * 128
    head_idx = np.zeros(C_HEAD, np.int64)
    head_valid = np.ones(C_HEAD, bool)
    head_idx[0:256] = np.arange(73984, 74240)          # b2 (tiles 0-1)
    head_idx[256:320] = np.arange(90624, 90688)        # b3 (tile 2 lo)
    head_valid[320:384] = False                        # pad (tile 2 hi)
    # W1'' tiles 3..68: tile 3+2i+h, partition p -> i<32: c=(128h+p)*32+i
    #                                               i=32: c=8192+128h+p
    t = np.arange(3, 69)
    i_of_t, h_of_t = (t - 3) // 2, (t - 3) % 2
    p = np.arange(128)
    cpos = (t[:, None] * 128 + p[None, :]).ravel()
    src = np.where(i_of_t[:, None] < 32,
                   (128 * h_of_t[:, None] + p[None, :]) * 32
                   + np.minimum(i_of_t[:, None], 31),
                   8192 + 128 * h_of_t[:, None] + p[None, :]).ravel()
    head_idx[cpos] = src

    w_head = np.zeros((C_HEAD, FEAT_CH), np.float32)
    b_head = np.zeros(C_HEAD, np.float32)
    w_head[head_valid] = w_feat[head_idx[head_valid]]
    b_head[head_valid] = b_feat[head_idx[head_valid]]

    b_all = np.concatenate([b_head, b_feat[8448:73984], b_feat[74240:90624]])
    bfeat_t = np.ascontiguousarray(b_all.reshape(N_TILES, 128).T)

    def wt(mat, dt, scale=1.0):
        # [C, 256] -> [128, 2, C]
        return np.ascontiguousarray(
            (mat.T * scale).reshape(2, 128, -1).transpose(1, 0, 2)).astype(dt)

    whead = wt(w_head, F16)
    w2q = wt(w_feat[8448:73984], E3, W2_SCALE)
    w3q = wt(w_feat[74240:90624], F16)

    wcdT = np.zeros((97, IN_CH), F16)
    wcdT[:OUT_CH] = w_cd.T.astype(F16)
    wcdT[OUT_CH] = b_cd.astype(F16)
    wcdT[65:97] = np.eye(IN_CH, dtype=F16)

    ones = np.ones((1, 4, S, 8, 64), F16)

    in_maps = []
    for core in range(N_CORES):
        b, hh = core // 2, core % 2
        mf = MLP_feature[b, :, 4 * hh:4 * hh + 4, :].reshape(2, 128, NQ)
        mlpfT = np.ascontiguousarray(mf.transpose(1, 0, 2)).astype(F16)
        mlpfs = np.ascontiguousarray(
            mf.transpose(1, 0, 2) / W2_SCALE).astype(F16)
        # cdem [97, r, s, ph, w]: rows 0-63 cd channels, 64 ones, 65-96 em
        cdem = np.empty((97, 4, S, 8, 64), F16)
        cd = coord_data[b].reshape(S, OUT_H, OUT_W, OUT_CH)[:, 32 * hh:32 * hh + 32]
        cdem[:OUT_CH] = (cd.transpose(3, 1, 0, 2)            # [o, h, s, w]
                         .reshape(OUT_CH, 4, 8, S, 64)
                         .transpose(0, 1, 3, 2, 4)).astype(F16)
        cdem[OUT_CH] = 1.0
        em = coord_em[b].reshape(IN_CH, S, OUT_H, OUT_W)[
            :, :, 32 * hh:32 * hh + 32, :]
        cdem[65:97] = (em.transpose(0, 2, 1, 3)              # [i, h, s, w]
                       .reshape(IN_CH, 4, 8, S, 64)
                       .transpose(0, 1, 3, 2, 4)).astype(F16)
        in_maps.append({
            "whead": whead, "w2": w2q, "w3": w3q, "bfeat": bfeat_t,
            "mlpfT": mlpfT, "mlpfs": mlpfs, "cdem": cdem, "wcdT": wcdT,
            "ones": ones,
        })
    return in_maps


def kernel(**inputs):
    inputs = {k: np.asarray(v) for k, v in inputs.items()}
    if "nc" not in _CACHE:
        _CACHE["nc"] = _build()
    nc = _CACHE["nc"]
    in_maps = _host_prep(**inputs)
    res = run_bass_kernel_spmd(nc, in_maps, core_ids=list(range(N_CORES)))
    out = np.empty((B, OUT_CH, S, OUT_H, OUT_W), np.float32)
    for core in range(N_CORES):
        b, hh = core // 2, core % 2
        o = res.results[core]["out"]                     # [64, 8, S, 8, 32]
        o = o.reshape(OUT_CH, 4, 2, S, 8, 32)            # (oc, r, h2, s, ph, w2)
        out[b, :, :, 32 * hh:32 * hh + 32, :] = (
            o.transpose(0, 3, 1, 4, 2, 5).reshape(OUT_CH, S, 32, OUT_W))
    return out


# revision 17
# speedup vs baseline: 1.0231x; 1.0231x over previous
"""Trainium2 Bass kernel for nn_PixelWiseAdpNet.

Sharding: (batch=4) x (patch-row-half=2) -> 8 cores; each core owns one
batch's 4x8 block of patches (32 patches, 16384 points).

v2 changes vs baseline (246us):
  - W2 region of w_feat streamed as fp8 e3m4 (x256 scale folded into a
    second copy of the patch-feature rhs); W1/W3/biases and all other
    tensors fp16 (not bf16) to buy numeric headroom for the fp8. The
    w_feat stream drops 46.5MB -> 29.7MB per core.
  - em merged into the coord-data matmul (augmented [w_cd.T; b_cd; I32]
    contraction, K=97) so inX needs one PSUM->SBUF copy, no add.
  - L1 bias folded into the matmul via a ones row (K=33; the W1''
    stream region interleaves b1 as hyper-channel i=32), so L1's
    LeakyReLU is bias-free and runs as ONE activation per patch pair.
  - Batched DMAs (>=4KB descriptors, r-major dram layouts for coords
    and output): ~45 triggers total vs ~170.
Stream order: [head fp16: b2 b3 W1''][W2 fp8e3][W3 fp16]; L1/L2h0/L2h1
interleave with the stream as their param regions complete; L3 after W3.
"""

import numpy as np
import ml_dtypes

import concourse.mybir as mybir
import concourse.tile as tile
from concourse import bacc
from concourse.bass_utils import run_bass_kernel_spmd
from concourse.masks import make_identity

F16 = np.float16
E3 = ml_dtypes.float8_e3m4

B, IN_CH, OUT_CH, FEAT_CH = 4, 32, 64, 256
AH = AW = 8
OUT_H = OUT_W = 64
S = 8
NEG = 0.01
N_CORES = 8
NQ = 32                  # patches per core

# stream tile layout (128-c tiles)
#   head (fp16): [b2:0-1][b3+pad:2][W1'':3-68]  (69 tiles, 8832 c)
#   w2   (fp8) : 512 tiles (65536 c)
#   w3   (fp16): 128 tiles (16384 c)
N_HEAD, N_W2, N_W3 = 69, 512, 128
N_TILES = N_HEAD + N_W2 + N_W3          # 709
T_W2, T_W3 = N_HEAD, N_HEAD + N_W2      # global region starts
W2_SCALE = 256.0

FP = mybir.dt.float16
F32 = mybir.dt.float32
F8 = mybir.dt.float8e3

_CACHE = {}


def _build(variant="all"):
    nreps = 1
    if variant.startswith("rep"):
        rep, _, rest = variant.partition(":")
        nreps = int(rep[3:])
        variant = rest or "all"
    nc = bacc.Bacc("TRN2", target_bir_lowering=False, debug=False,
                   num_devices=N_CORES)

    whead_d = nc.dram_tensor("whead", [128, 2, N_HEAD * 128], FP,
                             kind="ExternalInput")
    w2_d = nc.dram_tensor("w2", [128, 2, N_W2 * 128], F8, kind="ExternalInput")
    w3_d = nc.dram_tensor("w3", [128, 2, N_W3 * 128], FP, kind="ExternalInput")
    bfeat_d = nc.dram_tensor("bfeat", [128, N_TILES], F32, kind="ExternalInput")
    mlpfT_d = nc.dram_tensor("mlpfT", [128, 2, NQ], FP, kind="ExternalInput")
    mlpfs_d = nc.dram_tensor("mlpfs", [128, 2, NQ], FP, kind="ExternalInput")
    cdem_d = nc.dram_tensor("cdem", [97, 4, S, 8, 64], FP, kind="ExternalInput")
    wcdT_d = nc.dram_tensor("wcdT", [97, IN_CH], FP, kind="ExternalInput")
    ones_d = nc.dram_tensor("ones", [1, 4, S, 8, 64], FP, kind="ExternalInput")
    out_d = nc.dram_tensor("out", [OUT_CH, 8, S, 8, 32], F32,
                           kind="ExternalOutput")

    # slab schedule: (dram, global tile0, ntiles, scaled_rhs)
    slabs = []
    for t0 in range(0, N_HEAD, 16):
        slabs.append((whead_d, t0, min(16, N_HEAD - t0), False))
    for t0 in range(0, N_W2, 32):
        slabs.append((w2_d, T_W2 + t0, 32, True))
    for t0 in range(0, N_W3, 16):
        slabs.append((w3_d, T_W3 + t0, 16, False))
    N_SL_HEAD = (N_HEAD + 15) // 16                    # 5
    SL_W2_END = N_SL_HEAD + N_W2 // 32                 # 21
    N_SLABS = len(slabs)                               # 29
    region_t0 = {id(whead_d): 0, id(w2_d): T_W2, id(w3_d): T_W3}

    with tile.TileContext(nc) as tc:
        with (
            tc.tile_pool(name="const", bufs=1) as const_pool,
            tc.tile_pool(name="wstream", bufs=3) as wpool,
            tc.tile_pool(name="params", bufs=1) as ppool,
            tc.tile_pool(name="acts", bufs=3) as apool,
            tc.tile_pool(name="psA", bufs=2, space="PSUM") as psA,
        ):
            for _rep in range(nreps):
                # ---- constants ----
                ident = const_pool.tile([128, 128], FP, name="ident")
                make_identity(nc, ident[:])
                mlpfT = const_pool.tile([128, 2, NQ], FP, name="mlpfT")
                nc.scalar.dma_start(mlpfT[:], mlpfT_d[:])
                mlpfs = const_pool.tile([128, 2, NQ], FP, name="mlpfs")
                nc.scalar.dma_start(mlpfs[:], mlpfs_d[:])
                wcdT = const_pool.tile([97, IN_CH], FP, name="wcdT")
                nc.scalar.dma_start(wcdT[:], wcdT_d[:])
                bfeat = const_pool.tile([128, N_TILES], F32, name="bfeat")
                nc.scalar.dma_start(bfeat[:], bfeat_d[:])

                REG = [(0, T_W2), (T_W2, T_W3), (T_W3, N_TILES)]
                pregs = [
                    ppool.tile([128, N_HEAD, NQ], FP, name="pW1"),
                    ppool.tile([128, N_W2, NQ], FP, name="pW2"),
                    ppool.tile([128, N_W3, NQ], FP, name="pW3"),
                ]
                # bias_sb: 0,1 = b2 halves (head tiles 0,1); 2 = b3 (tile 2)
                bias_sb = ppool.tile([128, 3, NQ], F32, name="bias_sb")

                def preg(t):
                    for (lo, hi), pt in zip(REG, pregs):
                        if lo <= t < hi:
                            return pt, t - lo
                    raise AssertionError(t)

                if variant == "noA":
                    for pt in pregs:
                        nc.vector.memset(pt[:], 0.0)
                    nc.vector.memset(bias_sb[:], 0.0)
                n_slabs = 0 if variant == "noA" else N_SLABS

                pend = {}
                cursor = {"dma": 0, "go": 0}

                def slab_dma():
                    sl = cursor["dma"]
                    if sl >= n_slabs:
                        return
                    cursor["dma"] += 1
                    dram, t0, nt, _sc = slabs[sl]
                    cw = nt * 128
                    c0 = (t0 - region_t0[id(dram)]) * 128
                    dt = F8 if dram is w2_d else FP
                    wbuf = wpool.tile([128, 2, 4096 if dt is F8 else 2048],
                                      dt, name="wbuf")
                    nc.sync.dma_start(wbuf[:, :, :cw], dram[:, :, c0:c0 + cw])
                    pend[sl] = wbuf

                def slab_go():
                    sl = cursor["go"]
                    if sl >= n_slabs or sl not in pend:
                        return
                    cursor["go"] += 1
                    dram, t0, nt, scaled = slabs[sl]
                    wbuf = pend.pop(sl)
                    rhs = mlpfs if scaled else mlpfT
                    for ch0 in range(0, nt, 16):
                        chn = min(16, nt - ch0)
                        ps = psA.tile([128, 16, NQ], F32, name="ps")
                        for u in range(chn):
                            for k in range(2):
                                nc.tensor.matmul(
                                    ps[:, u, :],
                                    wbuf[:, k, (ch0 + u) * 128:(ch0 + u + 1) * 128],
                                    rhs[:, k, :],
                                    start=(k == 0), stop=(k == 1))
                        u = 0
                        while u < chn:
                            t = t0 + ch0 + u
                            pt, lt = preg(t)
                            seg = min(chn - u,
                                      next(hi for (lo, hi) in REG if lo <= t < hi) - t)
                            nc.vector.tensor_tensor(
                                out=pt[:, lt:lt + seg, :],
                                in0=ps[:, u:u + seg, :],
                                in1=bfeat[:, t:t + seg].unsqueeze(2).broadcast_to(
                                    (128, seg, NQ)),
                                op=mybir.AluOpType.add)
                            u += seg
                        for t in range(3):
                            if t0 + ch0 <= t < t0 + ch0 + chn:
                                nc.vector.tensor_scalar_add(
                                    bias_sb[:, t, :], ps[:, t - t0 - ch0, :],
                                    bfeat[:, t:t + 1])

                mlp_on = variant != "nomlp"
                x1s, x2h0s, x2h1s = {}, {}, {}

                with tc.tile_pool(name="early", bufs=1) as epool:
                    # ---- phase inX ----
                    x0 = epool.tile([33, 4, S, 8, 64], FP, name="x0")
                    nc.scalar.dma_start(x0[32:33, :, :, :, :], ones_d[:])
                    cdem_ts = {}

                    with tc.tile_pool(name="psE1", bufs=1,
                                      space="PSUM") as psE1:
                        def emit_inx(r, s):
                            cd_ps = psE1.tile([IN_CH, 8, 64], F32,
                                              name="cd_ps", tag="cd", bufs=2)
                            nc.tensor.matmul(cd_ps[:], wcdT[:],
                                             cdem_ts[r][:, s, :, :],
                                             start=True, stop=True)
                            nc.scalar.activation(
                                x0[:32, r, s, :, :], cd_ps[:],
                                mybir.ActivationFunctionType.Copy,
                                bias=0.0, scale=1.0)

                        for r in range(4):
                            t = epool.tile([97, S, 8, 64], FP, name="cdem_t",
                                           bufs=2)
                            nc.scalar.dma_start(t[:], cdem_d[:, r, :, :, :])
                            cdem_ts[r] = t

                        slab_dma()
                        for sl in range(N_SL_HEAD):
                            slab_dma()
                            if sl < 4:
                                for s in range(S):
                                    emit_inx(sl, s)
                            slab_go()

                    if mlp_on:
                        # W1'' fixup + L1 for all pairs; W2h0 stream drains
                        # in the gaps (8 slabs over 32 pairs)
                        with tc.tile_pool(name="psE2", bufs=1,
                                          space="PSUM") as psE2:
                            for q in range(NQ):
                                r, wp = q // 8, q % 8
                                w1T = epool.tile([33, 256], FP, name="w1T",
                                                 bufs=6)
                                tp = psE2.tile([33, 256], FP, name="tp",
                                               tag="tp", bufs=2)
                                for h in range(2):
                                    nc.tensor.transpose(
                                        tp[:, 128 * h:128 * h + 128],
                                        pregs[0][:, 3 + h:69:2, q],
                                        ident[:])
                                nc.vector.tensor_copy(w1T[:], tp[:])
                                xq = x0[:, r, :, :, 8 * wp:8 * wp + 8]
                                x1 = apool.tile([128, 2, 512], FP, name="x1",
                                                bufs=NQ)
                                for h in range(2):
                                    y1 = psE2.tile([128, 512], F32,
                                                   name="y1", tag="y", bufs=4)
                                    nc.tensor.matmul(
                                        y1[:], w1T[:, 128 * h:128 * h + 128],
                                        xq, start=True, stop=True)
                                    nc.scalar.activation(
                                        x1[:, h, :], y1[:],
                                        mybir.ActivationFunctionType.Lrelu,
                                        bias=0.0, scale=1.0, alpha=NEG)
                                x1s[q] = x1
                                if q % 4 == 3:
                                    slab_dma()
                                    slab_go()

                with (
                    tc.tile_pool(name="late", bufs=1) as lpool,
                    tc.tile_pool(name="psL", bufs=1, space="PSUM") as psL,
                ):
                    if mlp_on:
                        # L2 h=0 (W2 local tiles 0..255 ready); W2h1 stream
                        # drains in the gaps
                        for q in range(NQ):
                            x2 = lpool.tile([128, 512], FP, name="x2h0",
                                            bufs=NQ)
                            y2 = psL.tile([128, 512], F32, name="y2",
                                          tag="y", bufs=6)
                            for k in range(2):
                                nc.tensor.matmul(
                                    y2[:], pregs[1][:, k:k + 255:2, q],
                                    x1s[q][:, k, :], start=(k == 0),
                                    stop=(k == 1))
                            nc.scalar.activation(
                                x2[:], y2[:],
                                mybir.ActivationFunctionType.Lrelu,
                                bias=bias_sb[:, 0, q:q + 1], scale=1.0,
                                alpha=NEG)
                            x2h0s[q] = x2
                            if q % 4 == 1:
                                slab_dma()
                            if q % 4 == 3:
                                slab_go()
                    while cursor["go"] < SL_W2_END and cursor["go"] < n_slabs:
                        slab_dma()
                        slab_go()

                    H1E = 20

                    def emit_l2h1(q):
                        x2 = lpool.tile([128, 512], FP, name="x2h1", bufs=H1E)
                        y2 = psL.tile([128, 512], F32, name="y2", tag="y",
                                      bufs=6)
                        for k in range(2):
                            t0 = 256 + k
                            nc.tensor.matmul(
                                y2[:], pregs[1][:, t0:t0 + 255:2, q],
                                x1s[q][:, k, :], start=(k == 0), stop=(k == 1))
                        nc.scalar.activation(
                            x2[:], y2[:],
                            mybir.ActivationFunctionType.Lrelu,
                            bias=bias_sb[:, 1, q:q + 1], scale=1.0, alpha=NEG)
                        x2h1s[q] = x2

                    out_state = {}

                    def emit_l3(q):
                        r, wp = q // 8, q % 8
                        hr = 2 * r + wp // 4
                        if wp % 4 == 0:
                            out_state[hr] = lpool.tile([OUT_CH, S, 8, 32], F32,
                                                       name="out_row", bufs=2)
                        y3 = psL.tile([OUT_CH, 512], F32, name="y3", tag="y",
                                      bufs=6)
                        x2h = [x2h0s[q], x2h1s[q]]
                        for k in range(2):
                            nc.tensor.matmul(
                                y3[:], pregs[2][:, k:k + 127:2, q],
                                x2h[k][:], start=(k == 0), stop=(k == 1))
                        wo = (wp % 4) * 8
                        nc.vector.tensor_scalar_add(
                            out_state[hr][:, :, :, wo:wo + 8],
                            y3[:], bias_sb[:OUT_CH, 2, q:q + 1])
                        if wp % 4 == 3:
                            nc.sync.dma_start(out_d[:, hr, :, :, :],
                                              out_state[hr][:])

                    # W3 stream overlaps L2h1; L3 tail after W3 lands
                    for i in range(8):
                        slab_dma()
                        if mlp_on:
                            for q in range(2 * i, 2 * i + 2):
                                if q < H1E:
                                    emit_l2h1(q)
                        slab_go()
                    if mlp_on:
                        for q in range(16, H1E):
                            emit_l2h1(q)

                    if variant == "nomlp":
                        for hr in range(8):
                            out_row = lpool.tile([OUT_CH, S, 8, 32], F32,
                                                 name="out_rowM", bufs=2)
                            nc.vector.memset(out_row[:], 0.0)
                            nc.sync.dma_start(out_d[:, hr, :, :, :],
                                              out_row[:])
                    if mlp_on:
                        # L3(i) frees the x2h1 slot L2h1(i+H1E) needs
                        for q in range(H1E, NQ):
                            emit_l3(q - H1E)
                            emit_l2h1(q)
                        for q in range(NQ - H1E, NQ):
                            emit_l3(q)

    nc.compile()
    return nc


def _host_prep(MLP_feature, coord_em, coord_data, w_cd, b_cd, w_feat, b_feat):
    # orig c ranges: W1 0..8191 (c=o*32+i), b1 8192..8447, W2 8448..73983,
    # b2 73984..74239, W3 74240..90623, b3 90624..90687
    C_HEAD = N_HEAD * 128
    head_idx = np.zeros(C_HEAD, np.int64)
    head_valid = np.ones(C_HEAD, bool)
    head_idx[0:256] = np.arange(73984, 74240)          # b2 (tiles 0-1)
    head_idx[256:320] = np.arange(90624, 90688)        # b3 (tile 2 lo)
    head_valid[320:384] = False                        # pad (tile 2 hi)
    # W1'' tiles 3..68: tile 3+2i+h, partition p -> i<32: c=(128h+p)*32+i
    #                                               i=32: c=8192+128h+p
    t = np.arange(3, 69)
    i_of_t, h_of_t = (t - 3) // 2, (t - 3) % 2
    p = np.arange(128)
    cpos = (t[:, None] * 128 + p[None, :]).ravel()
    src = np.where(i_of_t[:, None] < 32,
                   (128 * h_of_t[:, None] + p[None, :]) * 32
                   + np.minimum(i_of_t[:, None], 31),
                   8192 + 128 * h_of_t[:, None] + p[None, :]).ravel()
    head_idx[cpos] = src

    w_head = np.zeros((C_HEAD, FEAT_CH), np.float32)
    b_head = np.zeros(C_HEAD, np.float32)
    w_head[head_valid] = w_feat[head_idx[head_valid]]
    b_head[head_valid] = b_feat[head_idx[head_valid]]

    b_all = np.concatenate([b_head, b_feat[8448:73984], b_feat[74240:90624]])
    bfeat_t = np.ascontiguousarray(b_all.reshape(N_TILES, 128).T)

    def wt(mat, dt, scale=1.0):
        # [C, 256] -> [128, 2, C]
        return np.ascontiguousarray(
            (mat.T * scale).reshape(2, 128, -1).transpose(1, 0, 2)).astype(dt)

    whead = wt(w_head, F16)
    w2q = wt(w_feat[8448:73984], E3, W2_SCALE)
    w3q = wt(w_feat[74240:90624], F16)

    wcdT = np.zeros((97, IN_CH), F16)
    wcdT[:OUT_CH] = w_cd.T.astype(F16)
    wcdT[OUT_CH] = b_cd.astype(F16)
    wcdT[65:97] = np.eye(IN_CH, dtype=F16)

    ones = np.ones((1, 4, S, 8, 64), F16)

    in_maps = []
    for core in range(N_CORES):
        b, hh = core // 2, core % 2
        mf = MLP_feature[b, :, 4 * hh:4 * hh + 4, :].reshape(2, 128, NQ)
        mlpfT = np.ascontiguousarray(mf.transpose(1, 0, 2)).astype(F16)
        mlpfs = np.ascontiguousarray(
            mf.transpose(1, 0, 2) / W2_SCALE).astype(F16)
        # cdem [97, r, s, ph, w]: rows 0-63 cd channels, 64 ones, 65-96 em
        cdem = np.empty((97, 4, S, 8, 64), F16)
        cd = coord_data[b].reshape(S, OUT_H, OUT_W, OUT_CH)[:, 32 * hh:32 * hh + 32]
        cdem[:OUT_CH] = (cd.transpose(3, 1, 0, 2)            # [o, h, s, w]
                         .reshape(OUT_CH, 4, 8, S, 64)
                         .transpose(0, 1, 3, 2, 4)).astype(F16)
        cdem[OUT_CH] = 1.0
        em = coord_em[b].reshape(IN_CH, S, OUT_H, OUT_W)[
            :, :, 32 * hh:32 * hh + 32, :]
        cdem[65:97] = (em.transpose(0, 2, 1, 3)              # [i, h, s, w]
                       .reshape(IN_CH, 4, 8, S, 64)
                       .transpose(0, 1, 3, 2, 4)).astype(F16)
        in_maps.append({
            "whead": whead, "w2": w2q, "w3": w3q, "bfeat": bfeat_t,
            "mlpfT": mlpfT, "mlpfs": mlpfs, "cdem": cdem, "wcdT": wcdT,
            "ones": ones,
        })
    return in_maps


def kernel(**inputs):
    inputs = {k: np.asarray(v) for k, v in inputs.items()}
    if "nc" not in _CACHE:
        _CACHE["nc"] = _build()
    nc = _CACHE["nc"]
    in_maps = _host_prep(**inputs)
    res = run_bass_kernel_spmd(nc, in_maps, core_ids=list(range(N_CORES)))
    out = np.empty((B, OUT_CH, S, OUT_H, OUT_W), np.float32)
    for core in range(N_CORES):
        b, hh = core // 2, core % 2
        o = res.results[core]["out"]                     # [64, 8, S, 8, 32]
        o = o.reshape(OUT_CH, 4, 2, S, 8, 32)            # (oc, r, h2, s, ph, w2)
        out[b, :, :, 32 * hh:32 * hh + 32, :] = (
            o.transpose(0, 3, 1, 4, 2, 5).reshape(OUT_CH, S, 32, OUT_W))
    return out
